# revision 19
# baseline (speedup 1.0000x reference)
"""Sparse-attention TRN2 kernel (v7).

Reference computation (per batch b):
  pf = normalize(x @ W_pf.T); ns = normalize(x @ W_ns.T); v = x @ W_v.T
  G = pf @ pf.T                                (T x T cosine sims)
  M[u, y] = max_{j<5} G[u, start(y)+j]         (sliding window max, clamped)
  S_pf[x, y] = sum_i w_pf[i] * M[start(x)+i, y]  == (W_band @ M)[x, y]
  S_ns[x, y] = sum_t Q[x, t] * (ns_n[t] . ns_n[y])   with
      Q[x, t] = sum_n w_ns[n] * [inxs[x, n] == t]    (host-precomputed)
  L = S_pf + S_ns + mask(radj);  attn = softmax(L, axis=-1);  out = attn @ v

Kernel computes L.T (y on partitions, x free) so softmax normalization and
the attn@v contraction need no transposes of the T x T tensors.

Performance structure (what profiling showed matters on this stack):
  - every dma_start job costs ~1.3us ring spin-up + ~40ns/KB per
    descriptor row, the two HWDGE rings' spin-ups serialize, and a ring is
    FIFO: so ALL inputs ride the sync ring as two fat blobs, critical
    [Wcat|xT] first; the 4 XBAR transposes and 2 output jobs follow FIFO
  - the NEFF has a fixed ~8.5us teardown (runtime clears all ~250 event
    semaphores, independent of kernel size) and ~2us of ring spin-up;
    only [first packet .. last output packet] is controllable
  - 4-deep per-batch software pipeline: per-batch norm reduce + rsqrt
    (rsqrt = Exp(-0.5*Ln) on the scalar engine -- both live in the
    natural_log_exp_and_others act table with Copy, see _patch_act_tables)
  - PE transposes replaced by [128,512]->[128,4,128] XBAR DMA transposes
  - adjacency mask = gpsimd multiply of exp(logits) by 0/1 rows (gpsimd is
    otherwise idle; this also keeps the logits PSUM chain 3 matmuls long)
"""

import sys

sys.path.insert(0, "/opt/trn_rl_repo")

from contextlib import ExitStack

import numpy as np

import concourse.bacc as bacc
import concourse.bass as bass
import concourse.tile as tile
from concourse import mybir
from concourse._compat import with_exitstack

B, T, C = 32, 256, 128
TNEI = 2
TOPK = 4
NEIGH = 2 * TNEI + 1
N_CORES = 8
BPC = B // N_CORES  # batches per core

F32 = mybir.dt.float32
I32 = mybir.dt.int32
BF16 = mybir.dt.bfloat16

Act = mybir.ActivationFunctionType
Alu = mybir.AluOpType

NP_BF16 = mybir.dt.np(BF16)

# fr blob: [Wcat (3C) | xTp-pair0 (2T) | xTp-pair1 (2T)]
FR_XT0 = 3 * C
FR_XT1 = 3 * C + 2 * T
FR_W = 3 * C + 4 * T

# rq blob: [WbT (2T) | EYE100 (C) | rq-pair0 (8T) | rq-pair1 (8T)]
RQ_WBT = 0
RQ_EYE = 2 * T
RQ_P0 = 2 * T + C
RQ_P1 = 10 * T + C
RQ_W = 18 * T + C


def _blk128(a2d):
    """(T, T)->(128, 2T): out[p, u*T+x] = a2d[x, u*128+p]."""
    return np.ascontiguousarray(
        a2d.T.reshape(2, 128, T).transpose(1, 0, 2).reshape(128, 2 * T)
    )


def host_prep(x, radj, inxs, W_pf, W_ns, W_v, v_pf, g_pf, v_ns, g_ns):
    """Build the two per-core input blobs (both ride the sync ring)."""
    w_pf = (g_pf[0] * v_pf / np.linalg.norm(v_pf)).astype(np.float32)
    w_ns = (g_ns[0] * v_ns / np.linalg.norm(v_ns)).astype(np.float32)

    start = np.clip(np.arange(T) - TNEI, 0, T - NEIGH)
    W_band = np.zeros((T, T), np.float32)
    for i in range(NEIGH):
        W_band[np.arange(T), start + i] = w_pf[i]
    WbT = _blk128(W_band).astype(NP_BF16)

    Wcat = np.concatenate([W_pf.T, W_ns.T, W_v.T], axis=1).astype(NP_BF16)
    eye100 = (np.eye(C, dtype=np.float32) * 100.0).astype(NP_BF16)

    xT = np.ascontiguousarray(x.transpose(0, 2, 1)).astype(NP_BF16)  # (B,C,T)
    radjT = np.stack(
        [_blk128((radj[i] != 0).astype(np.float32)) for i in range(B)]
    ).astype(NP_BF16)  # (B, 128, 2T), 1 kept / 0 masked
    rows = np.repeat(np.arange(T), TOPK)
    vals = np.tile(w_ns, T)
    QT = np.empty((B, 128, 2 * T), np.float32)
    for i in range(B):
        Q = np.zeros((T, T), np.float32)
        np.add.at(Q, (rows, np.asarray(inxs[i]).ravel()), vals)
        QT[i] = _blk128(Q)
    rqb = np.concatenate([radjT, QT.astype(NP_BF16)], axis=2)  # (B,128,4T)

    in_maps = []
    for core in range(N_CORES):
        b0 = core * BPC

        def pair(arr, pi):
            return np.concatenate(
                [arr[b0 + 2 * pi], arr[b0 + 2 * pi + 1]], axis=1
            )

        fr = np.concatenate([Wcat, pair(xT, 0), pair(xT, 1)], axis=1)
        rq = np.concatenate(
            [WbT, eye100, pair(rqb, 0), pair(rqb, 1)], axis=1
        )
        in_maps.append(
            dict(fr=np.ascontiguousarray(fr), rq=np.ascontiguousarray(rq))
        )
    return in_maps


@with_exitstack
def emit_kernel(ctx: ExitStack, tc: tile.TileContext, io: dict, bpc: int = BPC):
    nc = tc.nc
    W = 385  # per-token-block width of pjs: [pf(128) | ns(128) | v(128) | 1]

    inp = ctx.enter_context(tc.tile_pool(name="inp", bufs=1))
    work = ctx.enter_context(tc.tile_pool(name="work", bufs=4))
    pwork = ctx.enter_context(tc.tile_pool(name="pwork", bufs=3))
    small = ctx.enter_context(tc.tile_pool(name="small", bufs=4))
    outp = ctx.enter_context(tc.tile_pool(name="outp", bufs=2))
    ps_pj = ctx.enter_context(tc.tile_pool(name="ps_pj", bufs=3, space="PSUM"))
    ps_g = ctx.enter_context(tc.tile_pool(name="ps_g", bufs=2, space="PSUM"))
    ps_lt = ctx.enter_context(tc.tile_pool(name="ps_lt", bufs=1, space="PSUM"))
    ps_qn = ctx.enter_context(tc.tile_pool(name="ps_qn", bufs=1, space="PSUM"))

    # ---- the two input jobs, both on the sync ring, fr first ----
    fr = inp.tile([C, FR_W], BF16, name="fr")
    nc.sync.dma_start(fr[:], io["fr"][:])
    rq = inp.tile([128, RQ_W], BF16, name="rq")
    nc.sync.dma_start(rq[:], io["rq"][:])

    Wcat = fr[:, 0 : 3 * C]
    WbT0 = rq[:, RQ_WBT : RQ_WBT + T]
    WbT1 = rq[:, RQ_WBT + T : RQ_WBT + 2 * T]
    EYE100 = rq[:, RQ_EYE : RQ_EYE + C]

    B_ = [dict() for _ in range(bpc)]  # per-batch tile registry

    # -100 bias column for the fused-mask exp (const APs only ship 0/1)
    nbias = small.tile([128, 1], F32, name="nbias")
    with tc.tile_wait_until(0.0095):
        nc.gpsimd.memset(nbias[:], -100.0)

    def xt_blk(i, t):
        o = (FR_XT0 if i < 2 else FR_XT1) + (i % 2) * T
        return fr[:, o + t * C : o + (t + 1) * C]

    def radj_half(i, y):
        o = (RQ_P0 if i < 2 else RQ_P1) + (i % 2) * 4 * T
        return rq[:, o + y * T : o + (y + 1) * T]

    def qt_blk(i, t):
        o = (RQ_P0 if i < 2 else RQ_P1) + (i % 2) * 4 * T + 2 * T
        return rq[:, o + t * T : o + (t + 1) * T]

    # ---- front: proj, raw bf16 evac (with v|1 cols), squares+reduce ----
    def st_front(i, b):
        b["pj0"] = ps_pj.tile([128, 3 * C], F32, tag="pj", name=f"pj0_{i}")
        b["pj1"] = ps_pj.tile([128, 3 * C], F32, tag="pj", name=f"pj1_{i}")
        pj = (b["pj0"], b["pj1"])
        nc.tensor.matmul(pj[0][:], xt_blk(i, 0), Wcat, start=True, stop=True)
        nc.tensor.matmul(pj[1][:], xt_blk(i, 1), Wcat, start=True, stop=True)
        # raw bf16 copy of both projection blocks; col W-1 of each block is 1.0
        b["pjs"] = pjs = work.tile([128, 2 * W], BF16, tag="pjs", name=f"pjs{i}")
        nc.vector.tensor_copy(pjs[:, 0 : 3 * C], pj[0][:])
        nc.scalar.copy(pjs[:, W : W + 3 * C], pj[1][:])
        # no-dep memsets would otherwise schedule at t~6us and stretch the
        # measured kernel window; only needed before the out matmuls
        with tc.tile_wait_until(0.0095):
            nc.gpsimd.memset(
                bass.AP(
                    pjs.tensor, pjs.offset + 3 * C, [pjs.ap[0], [W, 2], [1, 1]]
                ),
                1.0,
            )
        # squares on gpsimd, per-batch row-sum on DVE
        b["sqs"] = sqs = pwork.tile([128, 4 * C], BF16, tag="sqs", name=f"sq{i}")
        for t in range(2):
            nc.gpsimd.tensor_tensor(
                sqs[:, 2 * t * C : 2 * (t + 1) * C],
                pjs[:, t * W : t * W + 2 * C],
                pjs[:, t * W : t * W + 2 * C],
                Alu.mult,
            )
        b["nrm2"] = nrm2 = small.tile([128, 4], F32, tag="nrm2", name=f"nr{i}")
        nc.vector.tensor_reduce(
            nrm2[:],
            bass.AP(sqs.tensor, sqs.offset, [sqs.ap[0], [C, 4], [1, C]]),
            mybir.AxisListType.X,
            Alu.add,
        )

    # nrm2/rinv col order: 2t + (0=pf,1=ns)
    # ---- per-batch rsqrt = exp(-ln/2) on the scalar engine ----
    def st_ftail(i, b):
        lg = small.tile([128, 4], F32, tag="lg", name=f"lg{i}")
        rinv = small.tile([128, 4], F32, tag="rinv", name=f"rinv{i}")
        b["rinv"] = rinv
        nc.scalar.activation(lg[:], b["nrm2"][:], Act.Ln)
        nc.scalar.activation(rinv[:], lg[:], Act.Exp, scale=-0.5)

    # ---- normalize on DVE, then XBAR transpose ----
    def st_norm(i, b):
        pjs, rinv = b["pjs"], b["rinv"]
        b["pnsn"] = pnsn = work.tile([128, 4 * C], BF16, tag="pnsn", name=f"pn{i}")
        for t in range(2):
            nc.vector.tensor_scalar(
                pnsn[:, t * C : (t + 1) * C],
                pjs[:, t * W : t * W + C],
                rinv[:, 2 * t : 2 * t + 1],
                None,
                Alu.mult,
            )
            nc.vector.tensor_scalar(
                pnsn[:, (2 + t) * C : (3 + t) * C],
                pjs[:, t * W + C : t * W + 2 * C],
                rinv[:, 2 * t + 1 : 2 * t + 2],
                None,
                Alu.mult,
            )

    def st_tpose(i, b):
        # pnsnT[c, j, t] = pnsn[t, j*128 + c]; j = {pf0, pf1, ns0, ns1}
        b["pnsnT"] = pT = work.tile([128, 4 * C], BF16, tag="pnsnT", name=f"pT{i}")
        nc.sync.dma_start_transpose(
            pT[:].rearrange("c (j t) -> c j t", j=4),
            b["pnsn"][:],
        )

    # ---- gram + evac, q ----
    def st_gram(i, b):
        G = ps_g.tile([128, 2 * T], F32, tag="G", name=f"G{i}")
        pT = b["pnsnT"]
        for u in range(2):
            nc.tensor.matmul(
                G[:, u * T : (u + 1) * T],
                pT[:, u * C : (u + 1) * C],
                pT[:, 0 : 2 * C],
                start=True,
                stop=True,
            )
        b["Gsb"] = Gsb = work.tile([128, 2 * T], BF16, tag="Gsb", name=f"Gs{i}")
        nc.scalar.copy(Gsb[:], G[:])

    def st_q(i, b):
        # q[c, x] = sum_t nsn[t, c] * Q[x, t]
        q = ps_qn.tile([C, T], F32, tag="qn", name=f"q{i}")
        pnsn = b["pnsn"]
        for t in range(2):
            nc.tensor.matmul(
                q[:],
                pnsn[:, (2 + t) * C : (3 + t) * C],
                qt_blk(i, t),
                start=(t == 0),
                stop=(t == 1),
            )
        b["qsb"] = qsb = work.tile([C, T], BF16, tag="qsb", name=f"qsb{i}")
        nc.scalar.copy(qsb[:], q[:])

    # ---- per-batch ladder: sliding-window max over both G blocks ----
    def st_ladder(i, b):
        Gsb = b["Gsb"]
        m1 = pwork.tile([128, 2 * T], BF16, tag="m1", name=f"m1_{i}")
        m2 = pwork.tile([128, 2 * T], BF16, tag="m2", name=f"m2_{i}")
        M = pwork.tile([128, 2 * T], BF16, tag="M", name=f"M{i}")
        b["M"] = M
        nc.vector.tensor_tensor(
            bass.AP(m1.tensor, m1.offset, [m1.ap[0], [T, 2], [1, T - 1]]),
            bass.AP(Gsb.tensor, Gsb.offset, [Gsb.ap[0], [T, 2], [1, T - 1]]),
            bass.AP(Gsb.tensor, Gsb.offset + 1, [Gsb.ap[0], [T, 2], [1, T - 1]]),
            Alu.max,
        )
        nc.vector.tensor_tensor(
            bass.AP(m2.tensor, m2.offset, [m2.ap[0], [T, 2], [1, T - 3]]),
            bass.AP(m1.tensor, m1.offset, [m1.ap[0], [T, 2], [1, T - 3]]),
            bass.AP(m1.tensor, m1.offset + 2, [m1.ap[0], [T, 2], [1, T - 3]]),
            Alu.max,
        )
        nc.vector.tensor_tensor(
            bass.AP(M.tensor, M.offset + 2, [M.ap[0], [T, 2], [1, T - 4]]),
            bass.AP(m2.tensor, m2.offset, [m2.ap[0], [T, 2], [1, T - 4]]),
            bass.AP(m1.tensor, m1.offset + 3, [m1.ap[0], [T, 2], [1, T - 4]]),
            Alu.max,
        )
        nc.gpsimd.tensor_copy(
            bass.AP(M.tensor, M.offset, [M.ap[0], [T, 2], [T - 2, 2], [1, 2]]),
            bass.AP(M.tensor, M.offset + 2, [M.ap[0], [T, 2], [251, 2], [0, 2]]),
        )

    # ---- logits (band + ns accumulated in PSUM), exp in y-halves ----
    # fuse_mask (last batch): accumulate 100*radj01 via an EYE100-stationary
    # matmul and exp with bias -100 -- drops the gpsimd mask from the
    # pipeline drain tail.
    def st_logits(i, b, fuse_mask=False):
        M = b["M"]
        LT = ps_lt.tile([128, 2 * T], F32, tag="LT", name=f"LT{i}")
        b["PTe"] = PTe = work.tile([128, 2 * T], BF16, tag="PTe", name=f"PTe{i}")
        for y in range(2):
            off = y * T
            if fuse_mask:
                nc.tensor.matmul(
                    LT[:, off : off + T], EYE100, radj_half(i, y),
                    start=True, stop=False,
                )
            nc.tensor.matmul(
                LT[:, off : off + T], M[:, y * C : (y + 1) * C], WbT0,
                start=not fuse_mask, stop=False,
            )
            nc.tensor.matmul(
                LT[:, off : off + T], M[:, T + y * C : T + (y + 1) * C], WbT1,
                start=False, stop=False,
            )
            nc.tensor.matmul(
                LT[:, off : off + T],
                b["pnsnT"][:, (2 + y) * C : (3 + y) * C],
                b["qsb"][:],
                start=False,
                stop=True,
            )
            nc.scalar.activation(
                PTe[:, off : off + T],
                LT[:, off : off + T],
                Act.Exp,
                bias=nbias[:] if fuse_mask else 0.0,
            )
        if fuse_mask:
            b["PT"] = PTe

    # ---- adjacency mask: 0/1 multiply on gpsimd, one op per y-half ----
    def st_mask(i, b):
        b["PT"] = PT = work.tile([128, 2 * T], BF16, tag="PT", name=f"PT{i}")
        for y in range(2):
            nc.gpsimd.tensor_tensor(
                PT[:, y * T : (y + 1) * T],
                b["PTe"][:, y * T : (y + 1) * T],
                radj_half(i, y),
                Alu.mult,
            )



    # ---- output ----
    def st_out(i, b, p):
        PT, pjs = b["PT"], b["pjs"]
        num = ps_qn.tile([128, 2 * (C + 1)], F32, tag="num", name=f"num{i}")
        for xt in range(2):
            osl = slice(xt * (C + 1), (xt + 1) * (C + 1))
            for y in range(2):
                nc.tensor.matmul(
                    num[:, osl],
                    PT[:, y * T + xt * C : y * T + (xt + 1) * C],
                    pjs[:, y * W + 2 * C : (y + 1) * W],
                    start=(y == 0),
                    stop=(y == 1),
                )
        dinv = small.tile([128, 2], F32, tag="dinv", name=f"dv{i}")
        nc.vector.reciprocal(
            dinv[:],
            bass.AP(num.tensor, num.offset + C, [num.ap[0], [C + 1, 2], [1, 1]]),
        )
        k = i % 2
        if k == 0:
            p["out_sb"] = outp.tile(
                [128, 2 * T], F32, tag="out_sb", name=f"o{i//2}"
            )
        out_sb = p["out_sb"]
        for xt in range(2):
            o = k * T + xt * C
            src = num[:, xt * (C + 1) : xt * (C + 1) + C]
            if xt == 0:
                nc.scalar.activation(
                    out_sb[:, o : o + C], src, Act.Copy, scale=dinv[:, 0:1]
                )
            else:
                nc.vector.tensor_scalar(
                    out_sb[:, o : o + C], src, dinv[:, 1:2], None, Alu.mult
                )

    def st_outdma(pi, p, half=None):
        out_sb = p["out_sb"]
        if half is None:
            od = io["out"][2 * pi]
            nc.sync.dma_start(
                bass.AP(od.tensor, od.offset, [[C, 128], [128 * C, 4], [1, C]]),
                bass.AP(
                    out_sb.tensor, out_sb.offset, [out_sb.ap[0], [C, 4], [1, C]]
                ),
            )
        else:
            od = io["out"][2 * pi + half]
            nc.sync.dma_start(
                bass.AP(od.tensor, od.offset, [[C, 128], [128 * C, 2], [1, C]]),
                bass.AP(
                    out_sb.tensor,
                    out_sb.offset + half * T,
                    [out_sb.ap[0], [C, 2], [1, C]],
                ),
            )

    P_ = [dict(), dict()]

    # ---- 4-deep per-batch software pipeline ----
    st_front(0, B_[0])
    st_front(1, B_[1])
    st_ftail(0, B_[0])
    st_norm(0, B_[0])
    st_tpose(0, B_[0])
    st_front(2, B_[2])
    st_ftail(1, B_[1])
    st_norm(1, B_[1])
    st_tpose(1, B_[1])
    st_gram(0, B_[0])
    st_q(0, B_[0])
    st_ladder(0, B_[0])
    st_front(3, B_[3])
    st_ftail(2, B_[2])
    st_norm(2, B_[2])
    st_tpose(2, B_[2])
    st_logits(0, B_[0])
    st_gram(1, B_[1])
    st_q(1, B_[1])
    st_ladder(1, B_[1])
    st_ftail(3, B_[3])
    st_norm(3, B_[3])
    st_tpose(3, B_[3])
    st_mask(0, B_[0])
    st_out(0, B_[0], P_[0])
    st_logits(1, B_[1])
    st_gram(2, B_[2])
    st_q(2, B_[2])
    st_ladder(2, B_[2])
    st_mask(1, B_[1])
    st_out(1, B_[1], P_[0])
    st_outdma(0, P_[0])
    st_logits(2, B_[2])
    st_gram(3, B_[3])
    st_q(3, B_[3])
    st_ladder(3, B_[3])
    st_mask(2, B_[2])
    st_out(2, B_[2], P_[1])
    st_outdma(1, P_[1], half=0)
    st_logits(3, B_[3], fuse_mask=True)
    st_out(3, B_[3], P_[1])
    st_outdma(1, P_[1], half=1)


def _patch_act_tables():
    """Keep only activation tables that cover ALL funcs this kernel uses
    (Copy/Ln/Exp), so insert_act_table_loads emits exactly one
    ACT_TABLE_LOAD instead of thrashing between tables (~1.5us/reload).
    act_func_set_id is positional, so every entry stays but non-target
    tables are blanked."""
    need = {Act.Copy, Act.Ln, Act.Exp}
    orig = bacc.get_activation_tables

    def filtered(module_arch):
        tabs = orig(module_arch)
        if not any(need <= v for v in tabs.values()):
            return tabs
        out, found = {}, False
        for k, v in tabs.items():
            if not found and need <= v:
                out[k] = v
                found = True
            else:
                out[k] = set()
        return out

    bacc.get_activation_tables = filtered


_patch_act_tables()


def build_nc(num_cores: int = 1, bpc: int = BPC):
    nc = bacc.Bacc(None, target_bir_lowering=False, debug=False)
    io = {
        "fr": nc.dram_tensor("fr", [C, FR_W], BF16, kind="ExternalInput"),
        "rq": nc.dram_tensor("rq", [128, RQ_W], BF16, kind="ExternalInput"),
        "out": nc.dram_tensor("out", [bpc, T, C], F32, kind="ExternalOutput"),
    }
    with tile.TileContext(nc, num_cores=num_cores) as tc:
        emit_kernel(tc, io, bpc=bpc)
    nc.compile()
    return nc


# ---------------------------------------------------------------------------
# Runner: full-input kernel() entry point.
# ---------------------------------------------------------------------------

_NC_CACHE = {}
LAST_RESULT = None


def _get_nc():
    if "nc" not in _NC_CACHE:
        _NC_CACHE["nc"] = build_nc(num_cores=N_CORES, bpc=BPC)
    return _NC_CACHE["nc"]


def _prep_in_maps(x, radj, inxs, W_pf, W_ns, W_v, v_pf, g_pf, v_ns, g_ns):
    return host_prep(
        np.asarray(x, np.float32),
        np.asarray(radj, np.int32),
        np.asarray(inxs),
        np.asarray(W_pf, np.float32),
        np.asarray(W_ns, np.float32),
        np.asarray(W_v, np.float32),
        np.asarray(v_pf, np.float32),
        np.asarray(g_pf, np.float32),
        np.asarray(v_ns, np.float32),
        np.asarray(g_ns, np.float32),
    )


def kernel(x, radj, inxs, W_pf, W_ns, W_v, v_pf, g_pf, v_ns, g_ns):
    global LAST_RESULT
    from concourse.bass_utils import run_bass_kernel_spmd

    in_maps = _prep_in_maps(
        x, radj, inxs, W_pf, W_ns, W_v, v_pf, g_pf, v_ns, g_ns
    )
    nc = _get_nc()
    res = run_bass_kernel_spmd(nc, in_maps, list(range(N_CORES)))
    LAST_RESULT = res
    out = np.concatenate([r["out"] for r in res.results], axis=0)
    return np.ascontiguousarray(out).astype(np.float32)
